# revision 54
# baseline (speedup 1.0000x reference)
"""Trainium2 Bass kernel for the nn_Dynamics problem.

Math (per batch element, d=8, H=128):
  x = X[:, :8], v = X[:, 8:]
  z0 = W0 x + b0; h0 = tanh(z0); z1 = W1 h0 + b1; h1 = tanh(z1)
  a1 = (1-h1^2)*w2;  A0 = W1^T a1;  a0 = (1-h0^2)*A0;  g = W0^T a0
  t0 = W0 v; h0p = (1-h0^2) t0; t1 = W1 h0p; u = h0 (1-h0^2) t0^2
  hvv = sum_h [-2*a1*h1*t1^2 - 2*A0*u]
  force = -(K x + D v)
  out = force - g * (g.force + hvv) / (1 + |g|^2)   (Sherman-Morrison)

Sign convention (saves ops; primed = negated): m0 = h0^2-1
  h0p' = m0 t0 = -h0p; t1' = -t1; u' = -u; a0' = -a0; g' = -g; e2' = A0 u' = -e2
  hvv = -2 sum(e1) + 2 sum(e2');  num = hvv - g'.p';  out = p' + num/(1+gg) * g'

Layout: features on partitions, batch on the free axis, tiles of 512.
The per-element scalar "tail" (dot products, Sherman-Morrison scale) runs
batch-major after a PE transpose of the packed [force; g; hvv] block.
All big matmuls use float32r (1 cycle/row vs 4 for fp32).

Sharding: pure data parallel over 8 NeuronCores (8192 rows each), weights
replicated, outputs concatenated.
"""

import os

import ml_dtypes
import numpy as np

import concourse.bacc as bacc
import concourse.bass as bass
import concourse.dve_ops as dve_ops
import concourse.tile as tile
from concourse import mybir
from concourse.bass_utils import run_bass_kernel_spmd
from concourse.dve_ops import DveOp
from concourse.dve_ops import has_src1
from concourse.dve_spec import C0, C1, One, Spec, Src0, Src1, lower, sq
from concourse.dve_uop import DveOpSpec
from concourse.masks import make_identity

F32 = mybir.dt.float32
F32R = mybir.dt.float32r
BF16 = mybir.dt.bfloat16
F16 = mybir.dt.float16
AX = mybir.AxisListType
OP = mybir.AluOpType
ACT = mybir.ActivationFunctionType

DIM = 8
H = 128
BATCH = 65536
NCORES = 8
BC = BATCH // NCORES          # 8192 rows per core
TW = 512                      # batch tile width
NT = BC // TW                 # 16 tiles per core
NCH = TW // 128               # 4 chunks of 128 per tile
JPC = BC // 128               # 64 column-groups in X_sb

LAST_RESULTS = None

# ---------------- custom fused DVE ops ----------------


def _register_op(name, body, reference):
    if name in dve_ops._SUB_OPCODE_FOR_NAME:
        for op in dve_ops.OPS:
            if op.name == name:
                return op
    spec = Spec(body=body, reference=reference)
    shas = {}
    for ver in ("v3", "v4"):
        shas[ver] = DveOpSpec(
            name=name,
            opcode=dve_ops._CUSTOM_DVE_ROW_BASE + len(dve_ops.OPS),
            uops=lower(spec, ver=ver),
            rd1_en=has_src1(spec),
        ).sha(ver)
    op = DveOp(name, spec, subdim=False, uops_sha=shas)
    dve_ops.OPS.append(op)
    dve_ops.CUSTOM_DVE_SPECS[name] = spec
    dve_ops._SUB_OPCODE_FOR_NAME[name] = (
        dve_ops._CUSTOM_DVE_ROW_BASE + len(dve_ops.OPS) - 1
    )
    return op


# h0p' = (h0^2 - 1) * t0     (also a0' = (h0^2 - 1) * A0)
OP_SQM1_MUL = _register_op(
    "ANT_SQM1_MUL",
    (sq(Src0) - One) * Src1,
    lambda in0, in1: (in0 * in0 - 1.0) * in1,
)
# u' = h0 * (h0^2 - 1) * t0^2
OP_UPRIME = _register_op(
    "ANT_UPRIME",
    Src0 * (sq(Src0) - One) * sq(Src1),
    lambda in0, in1: in0 * (in0 * in0 - 1.0) * in1 * in1,
)
# e1 = (1 - h1^2) * w2 * h1 * t1^2
OP_E1F = _register_op(
    "ANT_E1F",
    (One - sq(Src0)) * C0 * Src0 * sq(Src1),
    lambda in0, in1, s0: (1.0 - in0 * in0) * s0 * in0 * in1 * in1,
)
# a1 = (1 - h1^2) * w2
OP_A1F = _register_op(
    "ANT_A1F",
    (One - sq(Src0)) * C0,
    lambda in0, s0: (1.0 - in0 * in0) * s0,
)
# out = p' + (num * rec) * g'
OP_OUTF = _register_op(
    "ANT_OUTF",
    Src0 + (Src1 * C0) * C1,
    lambda in0, in1, s0, s1: in0 + (s0 * s1) * in1,
)


def build_nc():
    nc = bacc.Bacc()

    X = nc.dram_tensor("X", [BC, 2 * DIM], F32, kind="ExternalInput")
    # host-preprocessed weights (f32r where consumed by f32r matmuls)
    W0r = nc.dram_tensor("W0r", [H, 32], F16, kind="ExternalInput")
    W0Tx = nc.dram_tensor("W0Tx", [2 * DIM, H], F32R, kind="ExternalInput")
    W0Tv = nc.dram_tensor("W0Tv", [2 * DIM, H], F32R, kind="ExternalInput")
    W1 = nc.dram_tensor("W1", [H, H], F32R, kind="ExternalInput")
    W1T = nc.dram_tensor("W1T", [H, H], F32R, kind="ExternalInput")
    KDTn = nc.dram_tensor("KDTn", [2 * DIM, DIM], F32R, kind="ExternalInput")
    b0c = nc.dram_tensor("b0c", [H, 1], F32, kind="ExternalInput")
    b1c = nc.dram_tensor("b1c", [H, 1], F32, kind="ExternalInput")
    w2c = nc.dram_tensor("w2c", [H, 1], F32, kind="ExternalInput")
    out = nc.dram_tensor("out", [BC, DIM], F32, kind="ExternalOutput")

    from contextlib import ExitStack

    with tile.TileContext(nc) as tc, ExitStack() as stk:
        consts = stk.enter_context(tc.tile_pool(name="consts", bufs=1))
        work = stk.enter_context(tc.tile_pool(name="work", bufs=2))
        ps1 = stk.enter_context(tc.tile_pool(name="ps1", bufs=1, space="PSUM"))
        ps2 = stk.enter_context(tc.tile_pool(name="ps2", bufs=2, space="PSUM"))
        psT = ps1

        # ---------------- constants ----------------
        ident = consts.tile([128, 128], F32)
        make_identity(nc, ident)
        ident_r = consts.tile([128, 128], F32R)
        nc.scalar.copy(ident_r, ident)

        X_sb = consts.tile([128, JPC * 16], F32)
        nc.sync.dma_start(out=X_sb, in_=X.rearrange("(p j) f -> p (j f)", p=128))
        X_sr = consts.tile([128, JPC * 16], F32R)
        nc.scalar.copy(X_sr, X_sb)

        W0_sb = consts.tile([H, 32], F16)
        nc.sync.dma_start(out=W0_sb, in_=W0r[:, :])
        W0Tx_sb = consts.tile([2 * DIM, H], F32R)
        nc.sync.dma_start(out=W0Tx_sb, in_=W0Tx[:, :])
        W0Tv_sb = consts.tile([2 * DIM, H], F32R)
        nc.sync.dma_start(out=W0Tv_sb, in_=W0Tv[:, :])
        W1_sb = consts.tile([H, H], F32R)
        nc.sync.dma_start(out=W1_sb, in_=W1[:, :])
        W1T_sb = consts.tile([H, H], F32R)
        nc.sync.dma_start(out=W1T_sb, in_=W1T[:, :])
        KDTn_sb = consts.tile([2 * DIM, DIM], F32R)
        nc.sync.dma_start(out=KDTn_sb, in_=KDTn[:, :])
        b0_sb = consts.tile([H, 1], F32)
        nc.sync.dma_start(out=b0_sb, in_=b0c[:, :])
        b1_sb = consts.tile([H, 1], F32)
        nc.sync.dma_start(out=b1_sb, in_=b1c[:, :])
        w2_sb = consts.tile([H, 1], F32)
        nc.sync.dma_start(out=w2_sb, in_=w2c[:, :])

        # hvv reduction vectors: +-2 in cols 0:8, zeros in cols 8:32
        m2o8 = consts.tile([H, 32], F32)
        nc.vector.memset(m2o8, 0.0)
        nc.vector.memset(m2o8[:, 0:DIM], -2.0)
        m2o8_r = consts.tile([H, 32], F16)
        nc.scalar.copy(m2o8_r, m2o8)
        p2o8_r = consts.tile([H, 32], F16)
        nc.scalar.mul(p2o8_r, m2o8, -1.0)

        out_sb = consts.tile([128, JPC * DIM], F32)

        # ---------------- main loop ----------------
        for t in range(NT):
            # transpose 4 chunks of X into XT [16, 512] (features x batch)
            xt_ps = ps1.tile([2 * DIM, TW], F32R, tag="xt")
            for c in range(NCH):
                j = NCH * t + c
                if t == 0:
                    # fast start: don't wait for the X_sr rounding copy
                    nc.tensor.transpose(
                        xt_ps.bitcast(F32)[:, c * 128 : (c + 1) * 128],
                        X_sb[:, 16 * j : 16 * (j + 1)],
                        ident,
                    )
                else:
                    nc.tensor.transpose(
                        xt_ps[:, c * 128 : (c + 1) * 128],
                        X_sr[:, 16 * j : 16 * (j + 1)],
                        ident_r,
                    )
            XT = work.tile([2 * DIM, TW], F32R)
            nc.scalar.copy(XT, xt_ps.bitcast(F32))

            z0 = ps2.tile([H, TW], F32, tag="zz")
            nc.tensor.matmul(z0, W0Tx_sb, XT, start=True, stop=True)
            t0 = ps2.tile([H, TW], F32, tag="tt")
            nc.tensor.matmul(t0, W0Tv_sb, XT, start=True, stop=True)

            h0 = work.tile([H, TW], F32R)
            nc.scalar.activation(h0, z0, ACT.Tanh, bias=b0_sb, scale=1.0)
            h0f = h0.bitcast(F32)

            # h0p' = (h0^2-1)*t0 ; u' = h0*(h0^2-1)*t0^2
            h0p = work.tile([H, TW], F32R)
            nc.vector._custom_dve(OP_SQM1_MUL, out=h0p, in0=h0f, in1=t0[:, :])
            u = work.tile([H, TW], F32)
            nc.vector._custom_dve(OP_UPRIME, out=u, in0=h0f, in1=t0[:, :])

            z1 = ps2.tile([H, TW], F32, tag="zz")
            nc.tensor.matmul(z1, W1T_sb, h0, start=True, stop=True)
            t1 = ps2.tile([H, TW], F32, tag="tt")
            nc.tensor.matmul(t1, W1T_sb, h0p, start=True, stop=True)

            h1 = work.tile([H, TW], F32R)
            nc.scalar.activation(h1, z1, ACT.Tanh, bias=b1_sb, scale=1.0)
            h1f = h1.bitcast(F32)

            # a1 = (1-h1^2)*w2 ; e1 = a1*h1*t1^2
            a1 = work.tile([H, TW], F32R)
            nc.vector._custom_dve(OP_A1F, out=a1, in0=h1f, s0=w2_sb[:, 0:1])
            e1 = work.tile([H, TW], F16)
            nc.vector._custom_dve(
                OP_E1F, out=e1, in0=h1f, in1=t1[:, :], s0=w2_sb[:, 0:1]
            )

            A0 = ps1.tile([H, TW], F32, tag="A0")
            nc.tensor.matmul(A0, W1_sb, a1, start=True, stop=True)

            # a0' = (h0^2-1)*A0 ; e2' = A0*u'
            a0 = work.tile([H, TW], F16)
            nc.vector._custom_dve(OP_SQM1_MUL, out=a0, in0=h0f, in1=A0[:, :])
            e2 = work.tile([H, TW], F16)
            nc.vector.tensor_mul(e2, A0, u)

            # feature-major tail: p' rows 0:8, g' rows 32:40, hvv rows 64:72
            fm = psT.tile([96, TW], F32, tag="fm")
            nc.tensor.matmul(fm[0:DIM, :], KDTn_sb, XT, start=True, stop=True)
            nc.tensor.matmul(fm[32:64, :], W0_sb, a0, start=True, stop=True, tile_position=(0, 32))
            nc.tensor.matmul(fm[64:96, :], m2o8_r, e1, start=True, stop=False, tile_position=(0, 64))
            nc.tensor.matmul(fm[64:96, :], p2o8_r, e2, start=False, stop=True, tile_position=(0, 64))

            E = work.tile([72, TW], F32R)
            nc.scalar.copy(E, fm[0:72, :])

            # transpose to batch-major: bm[:, 72c + k] = E[k, 128c + p]
            bm = psT.tile([128, NCH * 72], F32R, tag="bm")
            for c in range(NCH):
                nc.tensor.transpose(
                    bm[:, 72 * c : 72 * (c + 1)],
                    E[:, c * 128 : (c + 1) * 128],
                    ident_r[0:72, 0:72],
                )
            # pack [p'(8) g'(8) hv(8)] per chunk: src rows 0:8,32:40,64:72
            tl = work.tile([128, NCH * 24], F32)
            bmf = bm.bitcast(F32)
            src4 = bass.AP(
                tensor=bmf.tensor,
                offset=bmf.offset,
                ap=[list(bmf.ap[0]), [72, NCH], [32, 3], [1, DIM]],
            )
            nc.scalar.copy(
                tl.rearrange("p (c q f) -> p c q f", q=3, f=DIM), src4
            )

            def col3(off, w):
                return bass.AP(
                    tensor=tl.tensor,
                    offset=tl.offset + off,
                    ap=[list(tl.ap[0]), [24, NCH], [1, w]],
                )

            p3 = col3(0, DIM)
            g3 = col3(DIM, DIM)
            hv2 = bass.AP(
                tensor=tl.tensor,
                offset=tl.offset + 2 * DIM,
                ap=[list(tl.ap[0]), [24, NCH]],
            )

            gb = work.tile([128, 2 * NCH * DIM], F32)
            gb3 = gb.rearrange("p (q c f) -> p (q c) f", f=DIM, q=2)
            nc.vector.tensor_mul(
                gb3[:, 0:NCH, :].rearrange("p c f -> p c f"), g3, g3
            )
            nc.vector.tensor_mul(
                gb3[:, NCH : 2 * NCH, :].rearrange("p c f -> p c f"), g3, p3
            )
            red = work.tile([128, 2 * NCH], F32)
            nc.vector.tensor_reduce(red, gb3, axis=AX.X, op=OP.add)
            gg = red[:, 0:NCH]
            gps = red[:, NCH : 2 * NCH]
            den = work.tile([128, NCH], F32)
            nc.vector.tensor_scalar_add(den, gg, 1.0)
            rec = work.tile([128, NCH], F32)
            nc.vector.reciprocal(rec, den)
            num = work.tile([128, NCH], F32)
            nc.vector.tensor_sub(num, hv2, gps)
            s4 = work.tile([128, NCH], F32)
            nc.vector.tensor_mul(s4, num, rec)
            s4b = bass.AP(
                tensor=s4.tensor,
                offset=s4.offset,
                ap=[list(s4.ap[0]), [1, NCH], [0, DIM]],
            )
            su = work.tile([128, NCH * DIM], F32)
            su3 = su.rearrange("p (c f) -> p c f", f=DIM)
            nc.vector.tensor_mul(su3, g3, s4b)
            ob = out_sb[:, DIM * NCH * t : DIM * NCH * (t + 1)]
            nc.vector.tensor_add(
                ob.rearrange("p (c f) -> p c f", f=DIM), p3, su3
            )

        nc.sync.dma_start(
            out=out.rearrange("(p j) f -> p (j f)", p=128), in_=out_sb
        )

    if not nc.is_finalized():
        nc.finalize()

    return nc


_NC_CACHE = None


def _install_ntff_shim():
    """Register the axon NTFF profile hook (missing antenv.axon_hooks shim)."""
    import sys
    import types

    if "antenv.axon_hooks" in sys.modules:
        return
    try:
        sys.path.insert(0, "/root/.axon_site")
        from trn_agent_boot.trn_boot import _ntff_profile_via_ctypes

        hook = _ntff_profile_via_ctypes("/opt/axon/libaxon_pjrt.so")
        mod = types.ModuleType("antenv.axon_hooks")
        mod.get_axon_ntff_profile_hook = lambda: hook
        sys.modules["antenv.axon_hooks"] = mod
    except Exception:
        pass


def kernel(**inputs):
    global LAST_RESULTS, _NC_CACHE
    trace = bool(int(os.environ.get("KERNEL_TRACE", "0")))
    if trace:
        _install_ntff_shim()
    if _NC_CACHE is None:
        _NC_CACHE = build_nc()
    nc = _NC_CACHE

    X = np.ascontiguousarray(inputs["X"], dtype=np.float32)
    K = np.asarray(inputs["K"], np.float32)
    D = np.asarray(inputs["D"], np.float32)
    W0 = np.asarray(inputs["W0"], np.float32)
    W1 = np.asarray(inputs["W1"], np.float32)
    W2 = np.asarray(inputs["W2"], np.float32)
    w0pad = np.zeros((H, 32), np.float32)
    w0pad[:, 0:DIM] = W0
    w0tx = np.zeros((2 * DIM, H), np.float32)
    w0tx[0:DIM] = W0.T
    w0tv = np.zeros((2 * DIM, H), np.float32)
    w0tv[DIM:] = W0.T
    shared = {
        "W0r": w0pad.astype(np.float16),
        "W0Tx": w0tx,
        "W0Tv": w0tv,
        "W1": np.ascontiguousarray(W1),
        "W1T": np.ascontiguousarray(W1.T),
        "KDTn": np.ascontiguousarray(np.concatenate([-K.T, -D.T], axis=0)),
        "b0c": np.asarray(inputs["b0"], np.float32).reshape(H, 1).copy(),
        "b1c": np.asarray(inputs["b1"], np.float32).reshape(H, 1).copy(),
        "w2c": W2.reshape(H, 1).copy(),
    }
    in_maps = []
    for i in range(NCORES):
        m = {"X": X[i * BC : (i + 1) * BC]}
        m.update(shared)
        in_maps.append(m)

    res = run_bass_kernel_spmd(
        nc, in_maps, core_ids=list(range(NCORES)), trace=trace
    )
    LAST_RESULTS = res
    out_full = np.concatenate([res.results[i]["out"] for i in range(NCORES)], axis=0)
    return out_full.astype(np.float32)


# revision 55
# speedup vs baseline: 1.0235x; 1.0235x over previous
"""Trainium2 Bass kernel for the nn_Dynamics problem.

Math (per batch element, d=8, H=128):
  x = X[:, :8], v = X[:, 8:]
  z0 = W0 x + b0; h0 = tanh(z0); z1 = W1 h0 + b1; h1 = tanh(z1)
  a1 = (1-h1^2)*w2;  A0 = W1^T a1;  a0 = (1-h0^2)*A0;  g = W0^T a0
  t0 = W0 v; h0p = (1-h0^2) t0; t1 = W1 h0p; u = h0 (1-h0^2) t0^2
  hvv = sum_h [-2*a1*h1*t1^2 - 2*A0*u]
  force = -(K x + D v)
  out = force - g * (g.force + hvv) / (1 + |g|^2)   (Sherman-Morrison)

Sign convention (saves ops; primed = negated): m0 = h0^2-1
  h0p' = m0 t0 = -h0p; t1' = -t1; u' = -u; a0' = -a0; g' = -g; e2' = A0 u' = -e2
  hvv = -2 sum(e1) + 2 sum(e2');  num = hvv - g'.p';  out = p' + num/(1+gg) * g'

Layout: features on partitions, batch on the free axis, tiles of 512.
The per-element scalar "tail" (dot products, Sherman-Morrison scale) runs
batch-major after a PE transpose of the packed [force; g; hvv] block.
All big matmuls use float32r (1 cycle/row vs 4 for fp32).

Sharding: pure data parallel over 8 NeuronCores (8192 rows each), weights
replicated, outputs concatenated.
"""

import os

import ml_dtypes
import numpy as np

import concourse.bacc as bacc
import concourse.bass as bass
import concourse.dve_ops as dve_ops
import concourse.tile as tile
from concourse import mybir
from concourse.bass_utils import run_bass_kernel_spmd
from concourse.dve_ops import DveOp
from concourse.dve_ops import has_src1
from concourse.dve_spec import C0, C1, One, Spec, Src0, Src1, lower, sq
from concourse.dve_uop import DveOpSpec
from concourse.masks import make_identity

F32 = mybir.dt.float32
F32R = mybir.dt.float32r
BF16 = mybir.dt.bfloat16
F16 = mybir.dt.float16
AX = mybir.AxisListType
OP = mybir.AluOpType
ACT = mybir.ActivationFunctionType

DIM = 8
H = 128
BATCH = 65536
NCORES = 8
BC = BATCH // NCORES          # 8192 rows per core
TW = 512                      # batch tile width
NT = BC // TW                 # 16 tiles per core
NCH = TW // 128               # 4 chunks of 128 per tile
JPC = BC // 128               # 64 column-groups in X_sb

LAST_RESULTS = None

# ---------------- custom fused DVE ops ----------------


def _register_op(name, body, reference):
    if name in dve_ops._SUB_OPCODE_FOR_NAME:
        for op in dve_ops.OPS:
            if op.name == name:
                return op
    spec = Spec(body=body, reference=reference)
    shas = {}
    for ver in ("v3", "v4"):
        shas[ver] = DveOpSpec(
            name=name,
            opcode=dve_ops._CUSTOM_DVE_ROW_BASE + len(dve_ops.OPS),
            uops=lower(spec, ver=ver),
            rd1_en=has_src1(spec),
        ).sha(ver)
    op = DveOp(name, spec, subdim=False, uops_sha=shas)
    dve_ops.OPS.append(op)
    dve_ops.CUSTOM_DVE_SPECS[name] = spec
    dve_ops._SUB_OPCODE_FOR_NAME[name] = (
        dve_ops._CUSTOM_DVE_ROW_BASE + len(dve_ops.OPS) - 1
    )
    return op


# h0p' = (h0^2 - 1) * t0     (also a0' = (h0^2 - 1) * A0)
OP_SQM1_MUL = _register_op(
    "ANT_SQM1_MUL",
    (sq(Src0) - One) * Src1,
    lambda in0, in1: (in0 * in0 - 1.0) * in1,
)
# u' = h0 * (h0^2 - 1) * t0^2
OP_UPRIME = _register_op(
    "ANT_UPRIME",
    Src0 * (sq(Src0) - One) * sq(Src1),
    lambda in0, in1: in0 * (in0 * in0 - 1.0) * in1 * in1,
)
# e1 = (1 - h1^2) * w2 * h1 * t1^2
OP_E1F = _register_op(
    "ANT_E1F",
    (One - sq(Src0)) * C0 * Src0 * sq(Src1),
    lambda in0, in1, s0: (1.0 - in0 * in0) * s0 * in0 * in1 * in1,
)
# a1 = (1 - h1^2) * w2
OP_A1F = _register_op(
    "ANT_A1F",
    (One - sq(Src0)) * C0,
    lambda in0, s0: (1.0 - in0 * in0) * s0,
)
# out = p' + (num * rec) * g'
OP_OUTF = _register_op(
    "ANT_OUTF",
    Src0 + (Src1 * C0) * C1,
    lambda in0, in1, s0, s1: in0 + (s0 * s1) * in1,
)


def build_nc():
    nc = bacc.Bacc()

    X = nc.dram_tensor("X", [BC, 2 * DIM], F32, kind="ExternalInput")
    # host-preprocessed weights (f32r where consumed by f32r matmuls)
    W0r = nc.dram_tensor("W0r", [H, 32], F16, kind="ExternalInput")
    W0Tx = nc.dram_tensor("W0Tx", [2 * DIM, H], F32R, kind="ExternalInput")
    W0Tv = nc.dram_tensor("W0Tv", [2 * DIM, H], F32R, kind="ExternalInput")
    W1 = nc.dram_tensor("W1", [H, H], F32R, kind="ExternalInput")
    W1T = nc.dram_tensor("W1T", [H, H], F32R, kind="ExternalInput")
    KDTn = nc.dram_tensor("KDTn", [2 * DIM, DIM], F32R, kind="ExternalInput")
    b0c = nc.dram_tensor("b0c", [H, 1], F32, kind="ExternalInput")
    b1c = nc.dram_tensor("b1c", [H, 1], F32, kind="ExternalInput")
    w2c = nc.dram_tensor("w2c", [H, 1], F32, kind="ExternalInput")
    out = nc.dram_tensor("out", [BC, DIM], F32, kind="ExternalOutput")

    from contextlib import ExitStack

    with tile.TileContext(nc) as tc, ExitStack() as stk:
        consts = stk.enter_context(tc.tile_pool(name="consts", bufs=1))
        work = stk.enter_context(tc.tile_pool(name="work", bufs=2))
        ps1 = stk.enter_context(tc.tile_pool(name="ps1", bufs=1, space="PSUM"))
        ps2 = stk.enter_context(tc.tile_pool(name="ps2", bufs=2, space="PSUM"))
        psT = ps1

        # ---------------- constants ----------------
        ident = consts.tile([128, 128], F32)
        make_identity(nc, ident)
        ident_r = consts.tile([128, 128], F32R)
        nc.scalar.copy(ident_r, ident)

        X_sb = consts.tile([128, JPC * 16], F32)
        nc.sync.dma_start(out=X_sb, in_=X.rearrange("(p j) f -> p (j f)", p=128))
        X_sr = consts.tile([128, JPC * 16], F32R)
        nc.scalar.copy(X_sr, X_sb)

        W0_sb = consts.tile([H, 32], F16)
        nc.sync.dma_start(out=W0_sb, in_=W0r[:, :])
        W0Tx_sb = consts.tile([2 * DIM, H], F32R)
        nc.sync.dma_start(out=W0Tx_sb, in_=W0Tx[:, :])
        W0Tv_sb = consts.tile([2 * DIM, H], F32R)
        nc.sync.dma_start(out=W0Tv_sb, in_=W0Tv[:, :])
        W1_sb = consts.tile([H, H], F32R)
        nc.sync.dma_start(out=W1_sb, in_=W1[:, :])
        W1T_sb = consts.tile([H, H], F32R)
        nc.sync.dma_start(out=W1T_sb, in_=W1T[:, :])
        KDTn_sb = consts.tile([2 * DIM, DIM], F32R)
        nc.sync.dma_start(out=KDTn_sb, in_=KDTn[:, :])
        b0_sb = consts.tile([H, 1], F32)
        nc.sync.dma_start(out=b0_sb, in_=b0c[:, :])
        b1_sb = consts.tile([H, 1], F32)
        nc.sync.dma_start(out=b1_sb, in_=b1c[:, :])
        w2_sb = consts.tile([H, 1], F32)
        nc.sync.dma_start(out=w2_sb, in_=w2c[:, :])

        # hvv reduction vectors: +-2 in cols 0:8, zeros in cols 8:32
        m2o8 = consts.tile([H, 32], F32)
        nc.vector.memset(m2o8, 0.0)
        nc.vector.memset(m2o8[:, 0:DIM], -2.0)
        m2o8_r = consts.tile([H, 32], F16)
        nc.scalar.copy(m2o8_r, m2o8)
        p2o8_r = consts.tile([H, 32], F16)
        nc.scalar.mul(p2o8_r, m2o8, -1.0)

        out_sb = consts.tile([128, JPC * DIM], F32)

        # ---------------- main loop ----------------
        for t in range(NT):
            # transpose 4 chunks of X into XT [16, 512] (features x batch)
            xt_ps = ps1.tile([2 * DIM, TW], F32R, tag="xt")
            for c in range(NCH):
                j = NCH * t + c
                nc.tensor.transpose(
                    xt_ps[:, c * 128 : (c + 1) * 128],
                    X_sr[:, 16 * j : 16 * (j + 1)],
                    ident_r,
                )
            XT = work.tile([2 * DIM, TW], F32R)
            nc.scalar.copy(XT, xt_ps.bitcast(F32))

            z0 = ps2.tile([H, TW], F32, tag="zz")
            nc.tensor.matmul(z0, W0Tx_sb, XT, start=True, stop=True)
            t0 = ps2.tile([H, TW], F32, tag="tt")
            nc.tensor.matmul(t0, W0Tv_sb, XT, start=True, stop=True)

            h0 = work.tile([H, TW], F32R)
            nc.scalar.activation(h0, z0, ACT.Tanh, bias=b0_sb, scale=1.0)
            h0f = h0.bitcast(F32)

            # h0p' = (h0^2-1)*t0 ; u' = h0*(h0^2-1)*t0^2
            h0p = work.tile([H, TW], F32R)
            nc.vector._custom_dve(OP_SQM1_MUL, out=h0p, in0=h0f, in1=t0[:, :])
            u = work.tile([H, TW], F32)
            nc.vector._custom_dve(OP_UPRIME, out=u, in0=h0f, in1=t0[:, :])

            z1 = ps2.tile([H, TW], F32, tag="zz")
            nc.tensor.matmul(z1, W1T_sb, h0, start=True, stop=True)
            t1 = ps2.tile([H, TW], F32, tag="tt")
            nc.tensor.matmul(t1, W1T_sb, h0p, start=True, stop=True)

            h1 = work.tile([H, TW], F32R)
            nc.scalar.activation(h1, z1, ACT.Tanh, bias=b1_sb, scale=1.0)
            h1f = h1.bitcast(F32)

            # a1 = (1-h1^2)*w2 ; e1 = a1*h1*t1^2
            a1 = work.tile([H, TW], F32R)
            nc.vector._custom_dve(OP_A1F, out=a1, in0=h1f, s0=w2_sb[:, 0:1])
            e1 = work.tile([H, TW], F16)
            nc.vector._custom_dve(
                OP_E1F, out=e1, in0=h1f, in1=t1[:, :], s0=w2_sb[:, 0:1]
            )

            A0 = ps1.tile([H, TW], F32, tag="A0")
            nc.tensor.matmul(A0, W1_sb, a1, start=True, stop=True)

            # a0' = (h0^2-1)*A0 ; e2' = A0*u'
            a0 = work.tile([H, TW], F16)
            nc.vector._custom_dve(OP_SQM1_MUL, out=a0, in0=h0f, in1=A0[:, :])
            e2 = work.tile([H, TW], F16)
            nc.vector.tensor_mul(e2, A0, u)

            # feature-major tail: p' rows 0:8, g' rows 32:40, hvv rows 64:72
            fm = psT.tile([96, TW], F32, tag="fm")
            nc.tensor.matmul(fm[0:DIM, :], KDTn_sb, XT, start=True, stop=True)
            nc.tensor.matmul(fm[32:64, :], W0_sb, a0, start=True, stop=True, tile_position=(0, 32))
            nc.tensor.matmul(fm[64:96, :], m2o8_r, e1, start=True, stop=False, tile_position=(0, 64))
            nc.tensor.matmul(fm[64:96, :], p2o8_r, e2, start=False, stop=True, tile_position=(0, 64))

            E = work.tile([72, TW], F32R)
            nc.scalar.copy(E, fm[0:72, :])

            # transpose to batch-major: bm[:, 72c + k] = E[k, 128c + p]
            bm = psT.tile([128, NCH * 72], F32R, tag="bm")
            for c in range(NCH):
                nc.tensor.transpose(
                    bm[:, 72 * c : 72 * (c + 1)],
                    E[:, c * 128 : (c + 1) * 128],
                    ident_r[0:72, 0:72],
                )
            # pack [p'(8) g'(8) hv(8)] per chunk: src rows 0:8,32:40,64:72
            tl = work.tile([128, NCH * 24], F32)
            bmf = bm.bitcast(F32)
            src4 = bass.AP(
                tensor=bmf.tensor,
                offset=bmf.offset,
                ap=[list(bmf.ap[0]), [72, NCH], [32, 3], [1, DIM]],
            )
            nc.scalar.copy(
                tl.rearrange("p (c q f) -> p c q f", q=3, f=DIM), src4
            )

            def col3(off, w):
                return bass.AP(
                    tensor=tl.tensor,
                    offset=tl.offset + off,
                    ap=[list(tl.ap[0]), [24, NCH], [1, w]],
                )

            p3 = col3(0, DIM)
            g3 = col3(DIM, DIM)
            hv2 = bass.AP(
                tensor=tl.tensor,
                offset=tl.offset + 2 * DIM,
                ap=[list(tl.ap[0]), [24, NCH]],
            )

            gb = work.tile([128, 2 * NCH * DIM], F32)
            gb3 = gb.rearrange("p (q c f) -> p (q c) f", f=DIM, q=2)
            nc.vector.tensor_mul(
                gb3[:, 0:NCH, :].rearrange("p c f -> p c f"), g3, g3
            )
            nc.vector.tensor_mul(
                gb3[:, NCH : 2 * NCH, :].rearrange("p c f -> p c f"), g3, p3
            )
            red = work.tile([128, 2 * NCH], F32)
            nc.vector.tensor_reduce(red, gb3, axis=AX.X, op=OP.add)
            gg = red[:, 0:NCH]
            gps = red[:, NCH : 2 * NCH]
            den = work.tile([128, NCH], F32)
            nc.vector.tensor_scalar_add(den, gg, 1.0)
            rec = work.tile([128, NCH], F32)
            nc.vector.reciprocal(rec, den)
            num = work.tile([128, NCH], F32)
            nc.vector.tensor_sub(num, hv2, gps)
            s4 = work.tile([128, NCH], F32)
            nc.vector.tensor_mul(s4, num, rec)
            s4b = bass.AP(
                tensor=s4.tensor,
                offset=s4.offset,
                ap=[list(s4.ap[0]), [1, NCH], [0, DIM]],
            )
            su = work.tile([128, NCH * DIM], F32)
            su3 = su.rearrange("p (c f) -> p c f", f=DIM)
            nc.vector.tensor_mul(su3, g3, s4b)
            ob = out_sb[:, DIM * NCH * t : DIM * NCH * (t + 1)]
            nc.vector.tensor_add(
                ob.rearrange("p (c f) -> p c f", f=DIM), p3, su3
            )

        nc.sync.dma_start(
            out=out.rearrange("(p j) f -> p (j f)", p=128), in_=out_sb
        )

    if not nc.is_finalized():
        nc.finalize()

    return nc


_NC_CACHE = None


def _install_ntff_shim():
    """Register the axon NTFF profile hook (missing antenv.axon_hooks shim)."""
    import sys
    import types

    if "antenv.axon_hooks" in sys.modules:
        return
    try:
        sys.path.insert(0, "/root/.axon_site")
        from trn_agent_boot.trn_boot import _ntff_profile_via_ctypes

        hook = _ntff_profile_via_ctypes("/opt/axon/libaxon_pjrt.so")
        mod = types.ModuleType("antenv.axon_hooks")
        mod.get_axon_ntff_profile_hook = lambda: hook
        sys.modules["antenv.axon_hooks"] = mod
    except Exception:
        pass


def kernel(**inputs):
    global LAST_RESULTS, _NC_CACHE
    trace = bool(int(os.environ.get("KERNEL_TRACE", "0")))
    if trace:
        _install_ntff_shim()
    if _NC_CACHE is None:
        _NC_CACHE = build_nc()
    nc = _NC_CACHE

    X = np.ascontiguousarray(inputs["X"], dtype=np.float32)
    K = np.asarray(inputs["K"], np.float32)
    D = np.asarray(inputs["D"], np.float32)
    W0 = np.asarray(inputs["W0"], np.float32)
    W1 = np.asarray(inputs["W1"], np.float32)
    W2 = np.asarray(inputs["W2"], np.float32)
    w0pad = np.zeros((H, 32), np.float32)
    w0pad[:, 0:DIM] = W0
    w0tx = np.zeros((2 * DIM, H), np.float32)
    w0tx[0:DIM] = W0.T
    w0tv = np.zeros((2 * DIM, H), np.float32)
    w0tv[DIM:] = W0.T
    shared = {
        "W0r": w0pad.astype(np.float16),
        "W0Tx": w0tx,
        "W0Tv": w0tv,
        "W1": np.ascontiguousarray(W1),
        "W1T": np.ascontiguousarray(W1.T),
        "KDTn": np.ascontiguousarray(np.concatenate([-K.T, -D.T], axis=0)),
        "b0c": np.asarray(inputs["b0"], np.float32).reshape(H, 1).copy(),
        "b1c": np.asarray(inputs["b1"], np.float32).reshape(H, 1).copy(),
        "w2c": W2.reshape(H, 1).copy(),
    }
    in_maps = []
    for i in range(NCORES):
        m = {"X": X[i * BC : (i + 1) * BC]}
        m.update(shared)
        in_maps.append(m)

    res = run_bass_kernel_spmd(
        nc, in_maps, core_ids=list(range(NCORES)), trace=trace
    )
    LAST_RESULTS = res
    out_full = np.concatenate([res.results[i]["out"] for i in range(NCORES)], axis=0)
    return out_full.astype(np.float32)


# revision 56
# speedup vs baseline: 1.0645x; 1.0401x over previous
"""Trainium2 Bass kernel for the nn_Dynamics problem.

Math (per batch element, d=8, H=128):
  x = X[:, :8], v = X[:, 8:]
  z0 = W0 x + b0; h0 = tanh(z0); z1 = W1 h0 + b1; h1 = tanh(z1)
  a1 = (1-h1^2)*w2;  A0 = W1^T a1;  a0 = (1-h0^2)*A0;  g = W0^T a0
  t0 = W0 v; h0p = (1-h0^2) t0; t1 = W1 h0p; u = h0 (1-h0^2) t0^2
  hvv = sum_h [-2*a1*h1*t1^2 - 2*A0*u]
  force = -(K x + D v)
  out = force - g * (g.force + hvv) / (1 + |g|^2)   (Sherman-Morrison)

Sign convention (saves ops; primed = negated): m0 = h0^2-1
  h0p' = m0 t0 = -h0p; t1' = -t1; u' = -u; a0' = -a0; g' = -g; e2' = A0 u' = -e2
  hvv = -2 sum(e1) + 2 sum(e2');  num = hvv - g'.p';  out = p' + num/(1+gg) * g'

Layout: features on partitions, batch on the free axis, tiles of 512.
The per-element scalar "tail" (dot products, Sherman-Morrison scale) runs
batch-major after a PE transpose of the packed [force; g; hvv] block.
All big matmuls use float32r (1 cycle/row vs 4 for fp32).

Sharding: pure data parallel over 8 NeuronCores (8192 rows each), weights
replicated, outputs concatenated.
"""

import os

import ml_dtypes
import numpy as np

import concourse.bacc as bacc
import concourse.bass as bass
import concourse.dve_ops as dve_ops
import concourse.tile as tile
from concourse import mybir
from concourse.bass_utils import run_bass_kernel_spmd
from concourse.dve_ops import DveOp
from concourse.dve_ops import has_src1
from concourse.dve_spec import C0, C1, One, Spec, Src0, Src1, lower, sq
from concourse.dve_uop import DveOpSpec
from concourse.masks import make_identity

F32 = mybir.dt.float32
F32R = mybir.dt.float32r
BF16 = mybir.dt.bfloat16
F16 = mybir.dt.float16
AX = mybir.AxisListType
OP = mybir.AluOpType
ACT = mybir.ActivationFunctionType

DIM = 8
H = 128
BATCH = 65536
NCORES = 8
BC = BATCH // NCORES          # 8192 rows per core
TW = 512                      # batch tile width
NT = BC // TW                 # 16 tiles per core
NCH = TW // 128               # 4 chunks of 128 per tile
JPC = BC // 128               # 64 column-groups in X_sb

LAST_RESULTS = None

# ---------------- custom fused DVE ops ----------------


def _register_op(name, body, reference):
    if name in dve_ops._SUB_OPCODE_FOR_NAME:
        for op in dve_ops.OPS:
            if op.name == name:
                return op
    spec = Spec(body=body, reference=reference)
    shas = {}
    for ver in ("v3", "v4"):
        shas[ver] = DveOpSpec(
            name=name,
            opcode=dve_ops._CUSTOM_DVE_ROW_BASE + len(dve_ops.OPS),
            uops=lower(spec, ver=ver),
            rd1_en=has_src1(spec),
        ).sha(ver)
    op = DveOp(name, spec, subdim=False, uops_sha=shas)
    dve_ops.OPS.append(op)
    dve_ops.CUSTOM_DVE_SPECS[name] = spec
    dve_ops._SUB_OPCODE_FOR_NAME[name] = (
        dve_ops._CUSTOM_DVE_ROW_BASE + len(dve_ops.OPS) - 1
    )
    return op


# h0p' = (h0^2 - 1) * t0     (also a0' = (h0^2 - 1) * A0)
OP_SQM1_MUL = _register_op(
    "ANT_SQM1_MUL",
    (sq(Src0) - One) * Src1,
    lambda in0, in1: (in0 * in0 - 1.0) * in1,
)
# u' = h0 * (h0^2 - 1) * t0^2
OP_UPRIME = _register_op(
    "ANT_UPRIME",
    Src0 * (sq(Src0) - One) * sq(Src1),
    lambda in0, in1: in0 * (in0 * in0 - 1.0) * in1 * in1,
)
# e1 = (1 - h1^2) * w2 * h1 * t1^2
OP_E1F = _register_op(
    "ANT_E1F",
    (One - sq(Src0)) * C0 * Src0 * sq(Src1),
    lambda in0, in1, s0: (1.0 - in0 * in0) * s0 * in0 * in1 * in1,
)
# a1 = (1 - h1^2) * w2
OP_A1F = _register_op(
    "ANT_A1F",
    (One - sq(Src0)) * C0,
    lambda in0, s0: (1.0 - in0 * in0) * s0,
)
# out = p' + (num * rec) * g'
OP_OUTF = _register_op(
    "ANT_OUTF",
    Src0 + (Src1 * C0) * C1,
    lambda in0, in1, s0, s1: in0 + (s0 * s1) * in1,
)


def build_nc():
    nc = bacc.Bacc()

    X = nc.dram_tensor("X", [BC, 2 * DIM], F32, kind="ExternalInput")
    # host-preprocessed weights (f32r where consumed by f32r matmuls)
    W0r = nc.dram_tensor("W0r", [H, 32], F16, kind="ExternalInput")
    W0Tx = nc.dram_tensor("W0Tx", [2 * DIM, H], F32R, kind="ExternalInput")
    W0Tv = nc.dram_tensor("W0Tv", [2 * DIM, H], F32R, kind="ExternalInput")
    W1 = nc.dram_tensor("W1", [H, H], F16, kind="ExternalInput")
    W1T = nc.dram_tensor("W1T", [H, H], F16, kind="ExternalInput")
    KDTn = nc.dram_tensor("KDTn", [2 * DIM, DIM], F32R, kind="ExternalInput")
    b0c = nc.dram_tensor("b0c", [H, 1], F32, kind="ExternalInput")
    b1c = nc.dram_tensor("b1c", [H, 1], F32, kind="ExternalInput")
    w2c = nc.dram_tensor("w2c", [H, 1], F32, kind="ExternalInput")
    out = nc.dram_tensor("out", [BC, DIM], F32, kind="ExternalOutput")

    from contextlib import ExitStack

    with tile.TileContext(nc) as tc, ExitStack() as stk:
        consts = stk.enter_context(tc.tile_pool(name="consts", bufs=1))
        work = stk.enter_context(tc.tile_pool(name="work", bufs=2))
        ps1 = stk.enter_context(tc.tile_pool(name="ps1", bufs=1, space="PSUM"))
        ps2 = stk.enter_context(tc.tile_pool(name="ps2", bufs=2, space="PSUM"))
        psT = ps1

        # ---------------- constants ----------------
        ident = consts.tile([128, 128], F32)
        make_identity(nc, ident)
        ident_r = consts.tile([128, 128], F32R)
        nc.scalar.copy(ident_r, ident)

        X_sb = consts.tile([128, JPC * 16], F32)
        nc.sync.dma_start(out=X_sb, in_=X.rearrange("(p j) f -> p (j f)", p=128))
        X_sr = consts.tile([128, JPC * 16], F32R)
        nc.scalar.copy(X_sr, X_sb)

        W0_sb = consts.tile([H, 32], F16)
        nc.sync.dma_start(out=W0_sb, in_=W0r[:, :])
        W0Tx_sb = consts.tile([2 * DIM, H], F32R)
        nc.sync.dma_start(out=W0Tx_sb, in_=W0Tx[:, :])
        W0Tv_sb = consts.tile([2 * DIM, H], F32R)
        nc.sync.dma_start(out=W0Tv_sb, in_=W0Tv[:, :])
        W1_sb = consts.tile([H, H], F16)
        nc.sync.dma_start(out=W1_sb, in_=W1[:, :])
        W1T_sb = consts.tile([H, H], F16)
        nc.sync.dma_start(out=W1T_sb, in_=W1T[:, :])
        KDTn_sb = consts.tile([2 * DIM, DIM], F32R)
        nc.sync.dma_start(out=KDTn_sb, in_=KDTn[:, :])
        b0_sb = consts.tile([H, 1], F32)
        nc.sync.dma_start(out=b0_sb, in_=b0c[:, :])
        b1_sb = consts.tile([H, 1], F32)
        nc.sync.dma_start(out=b1_sb, in_=b1c[:, :])
        w2_sb = consts.tile([H, 1], F32)
        nc.sync.dma_start(out=w2_sb, in_=w2c[:, :])

        # hvv reduction vectors: +-2 in cols 0:8, zeros in cols 8:32
        m2o8 = consts.tile([H, 32], F32)
        nc.vector.memset(m2o8, 0.0)
        nc.vector.memset(m2o8[:, 0:DIM], -2.0)
        m2o8_r = consts.tile([H, 32], F16)
        nc.scalar.copy(m2o8_r, m2o8)
        p2o8_r = consts.tile([H, 32], F16)
        nc.scalar.mul(p2o8_r, m2o8, -1.0)

        out_sb = consts.tile([128, JPC * DIM], F32)

        # ---------------- main loop ----------------
        for t in range(NT):
            # transpose 4 chunks of X into XT [16, 512] (features x batch)
            xt_ps = ps1.tile([2 * DIM, TW], F32R, tag="xt")
            for c in range(NCH):
                j = NCH * t + c
                nc.tensor.transpose(
                    xt_ps[:, c * 128 : (c + 1) * 128],
                    X_sr[:, 16 * j : 16 * (j + 1)],
                    ident_r,
                )
            XT = work.tile([2 * DIM, TW], F32R)
            nc.scalar.copy(XT, xt_ps.bitcast(F32))

            z0 = ps2.tile([H, TW], F32, tag="zz")
            nc.tensor.matmul(z0, W0Tx_sb, XT, start=True, stop=True)
            t0 = ps2.tile([H, TW], F32, tag="tt")
            nc.tensor.matmul(t0, W0Tv_sb, XT, start=True, stop=True)

            h0 = work.tile([H, TW], F16)
            nc.scalar.activation(h0, z0, ACT.Tanh, bias=b0_sb, scale=1.0)
            h0f = h0

            # h0p' = (h0^2-1)*t0 ; u' = h0*(h0^2-1)*t0^2
            h0p = work.tile([H, TW], F16)
            nc.vector._custom_dve(OP_SQM1_MUL, out=h0p, in0=h0f, in1=t0[:, :])
            u = work.tile([H, TW], F32)
            nc.vector._custom_dve(OP_UPRIME, out=u, in0=h0f, in1=t0[:, :])

            z1 = ps2.tile([H, TW], F32, tag="zz")
            nc.tensor.matmul(z1, W1T_sb, h0, start=True, stop=True)
            t1 = ps2.tile([H, TW], F32, tag="tt")
            nc.tensor.matmul(t1, W1T_sb, h0p, start=True, stop=True)

            h1 = work.tile([H, TW], F32R)
            nc.scalar.activation(h1, z1, ACT.Tanh, bias=b1_sb, scale=1.0)
            h1f = h1.bitcast(F32)

            # a1 = (1-h1^2)*w2 ; e1 = a1*h1*t1^2
            a1 = work.tile([H, TW], F16)
            nc.vector._custom_dve(OP_A1F, out=a1, in0=h1f, s0=w2_sb[:, 0:1])
            e1 = work.tile([H, TW], F16)
            nc.vector._custom_dve(
                OP_E1F, out=e1, in0=h1f, in1=t1[:, :], s0=w2_sb[:, 0:1]
            )

            A0 = ps1.tile([H, TW], F32, tag="A0")
            nc.tensor.matmul(A0, W1_sb, a1, start=True, stop=True)

            # a0' = (h0^2-1)*A0 ; e2' = A0*u'
            a0 = work.tile([H, TW], F16)
            nc.vector._custom_dve(OP_SQM1_MUL, out=a0, in0=h0f, in1=A0[:, :])
            e2 = work.tile([H, TW], F16)
            nc.vector.tensor_mul(e2, A0, u)

            # feature-major tail: p' rows 0:8, g' rows 32:40, hvv rows 64:72
            fm = psT.tile([96, TW], F32, tag="fm")
            nc.tensor.matmul(fm[0:DIM, :], KDTn_sb, XT, start=True, stop=True)
            nc.tensor.matmul(fm[32:64, :], W0_sb, a0, start=True, stop=True, tile_position=(0, 32))
            nc.tensor.matmul(fm[64:96, :], m2o8_r, e1, start=True, stop=False, tile_position=(0, 64))
            nc.tensor.matmul(fm[64:96, :], p2o8_r, e2, start=False, stop=True, tile_position=(0, 64))

            E = work.tile([72, TW], F32R)
            nc.scalar.copy(E, fm[0:72, :])

            # transpose to batch-major: bm[:, 72c + k] = E[k, 128c + p]
            bm = psT.tile([128, NCH * 72], F32R, tag="bm")
            for c in range(NCH):
                nc.tensor.transpose(
                    bm[:, 72 * c : 72 * (c + 1)],
                    E[:, c * 128 : (c + 1) * 128],
                    ident_r[0:72, 0:72],
                )
            # pack [p'(8) g'(8) hv(8)] per chunk: src rows 0:8,32:40,64:72
            tl = work.tile([128, NCH * 24], F32)
            bmf = bm.bitcast(F32)
            src4 = bass.AP(
                tensor=bmf.tensor,
                offset=bmf.offset,
                ap=[list(bmf.ap[0]), [72, NCH], [32, 3], [1, DIM]],
            )
            nc.scalar.copy(
                tl.rearrange("p (c q f) -> p c q f", q=3, f=DIM), src4
            )

            def col3(off, w):
                return bass.AP(
                    tensor=tl.tensor,
                    offset=tl.offset + off,
                    ap=[list(tl.ap[0]), [24, NCH], [1, w]],
                )

            p3 = col3(0, DIM)
            g3 = col3(DIM, DIM)
            hv2 = bass.AP(
                tensor=tl.tensor,
                offset=tl.offset + 2 * DIM,
                ap=[list(tl.ap[0]), [24, NCH]],
            )

            gb = work.tile([128, 2 * NCH * DIM], F32)
            gb3 = gb.rearrange("p (q c f) -> p (q c) f", f=DIM, q=2)
            nc.vector.tensor_mul(
                gb3[:, 0:NCH, :].rearrange("p c f -> p c f"), g3, g3
            )
            nc.vector.tensor_mul(
                gb3[:, NCH : 2 * NCH, :].rearrange("p c f -> p c f"), g3, p3
            )
            red = work.tile([128, 2 * NCH], F32)
            nc.vector.tensor_reduce(red, gb3, axis=AX.X, op=OP.add)
            gg = red[:, 0:NCH]
            gps = red[:, NCH : 2 * NCH]
            den = work.tile([128, NCH], F32)
            nc.vector.tensor_scalar_add(den, gg, 1.0)
            rec = work.tile([128, NCH], F32)
            nc.vector.reciprocal(rec, den)
            num = work.tile([128, NCH], F32)
            nc.vector.tensor_sub(num, hv2, gps)
            s4 = work.tile([128, NCH], F32)
            nc.vector.tensor_mul(s4, num, rec)
            s4b = bass.AP(
                tensor=s4.tensor,
                offset=s4.offset,
                ap=[list(s4.ap[0]), [1, NCH], [0, DIM]],
            )
            su = work.tile([128, NCH * DIM], F32)
            su3 = su.rearrange("p (c f) -> p c f", f=DIM)
            nc.vector.tensor_mul(su3, g3, s4b)
            ob = out_sb[:, DIM * NCH * t : DIM * NCH * (t + 1)]
            nc.vector.tensor_add(
                ob.rearrange("p (c f) -> p c f", f=DIM), p3, su3
            )

        nc.sync.dma_start(
            out=out.rearrange("(p j) f -> p (j f)", p=128), in_=out_sb
        )

    if not nc.is_finalized():
        nc.finalize()

    return nc


_NC_CACHE = None


def _install_ntff_shim():
    """Register the axon NTFF profile hook (missing antenv.axon_hooks shim)."""
    import sys
    import types

    if "antenv.axon_hooks" in sys.modules:
        return
    try:
        sys.path.insert(0, "/root/.axon_site")
        from trn_agent_boot.trn_boot import _ntff_profile_via_ctypes

        hook = _ntff_profile_via_ctypes("/opt/axon/libaxon_pjrt.so")
        mod = types.ModuleType("antenv.axon_hooks")
        mod.get_axon_ntff_profile_hook = lambda: hook
        sys.modules["antenv.axon_hooks"] = mod
    except Exception:
        pass


def kernel(**inputs):
    global LAST_RESULTS, _NC_CACHE
    trace = bool(int(os.environ.get("KERNEL_TRACE", "0")))
    if trace:
        _install_ntff_shim()
    if _NC_CACHE is None:
        _NC_CACHE = build_nc()
    nc = _NC_CACHE

    X = np.ascontiguousarray(inputs["X"], dtype=np.float32)
    K = np.asarray(inputs["K"], np.float32)
    D = np.asarray(inputs["D"], np.float32)
    W0 = np.asarray(inputs["W0"], np.float32)
    W1 = np.asarray(inputs["W1"], np.float32)
    W2 = np.asarray(inputs["W2"], np.float32)
    w0pad = np.zeros((H, 32), np.float32)
    w0pad[:, 0:DIM] = W0
    w0tx = np.zeros((2 * DIM, H), np.float32)
    w0tx[0:DIM] = W0.T
    w0tv = np.zeros((2 * DIM, H), np.float32)
    w0tv[DIM:] = W0.T
    shared = {
        "W0r": w0pad.astype(np.float16),
        "W0Tx": w0tx,
        "W0Tv": w0tv,
        "W1": W1.astype(np.float16),
        "W1T": np.ascontiguousarray(W1.T).astype(np.float16),
        "KDTn": np.ascontiguousarray(np.concatenate([-K.T, -D.T], axis=0)),
        "b0c": np.asarray(inputs["b0"], np.float32).reshape(H, 1).copy(),
        "b1c": np.asarray(inputs["b1"], np.float32).reshape(H, 1).copy(),
        "w2c": W2.reshape(H, 1).copy(),
    }
    in_maps = []
    for i in range(NCORES):
        m = {"X": X[i * BC : (i + 1) * BC]}
        m.update(shared)
        in_maps.append(m)

    res = run_bass_kernel_spmd(
        nc, in_maps, core_ids=list(range(NCORES)), trace=trace
    )
    LAST_RESULTS = res
    out_full = np.concatenate([res.results[i]["out"] for i in range(NCORES)], axis=0)
    return out_full.astype(np.float32)


# revision 57
# speedup vs baseline: 1.1284x; 1.0600x over previous
"""Trainium2 Bass kernel for the nn_Dynamics problem.

Math (per batch element, d=8, H=128):
  x = X[:, :8], v = X[:, 8:]
  z0 = W0 x + b0; h0 = tanh(z0); z1 = W1 h0 + b1; h1 = tanh(z1)
  a1 = (1-h1^2)*w2;  A0 = W1^T a1;  a0 = (1-h0^2)*A0;  g = W0^T a0
  t0 = W0 v; h0p = (1-h0^2) t0; t1 = W1 h0p; u = h0 (1-h0^2) t0^2
  hvv = sum_h [-2*a1*h1*t1^2 - 2*A0*u]
  force = -(K x + D v)
  out = force - g * (g.force + hvv) / (1 + |g|^2)   (Sherman-Morrison)

Sign convention (saves ops; primed = negated): m0 = h0^2-1
  h0p' = m0 t0 = -h0p; t1' = -t1; u' = -u; a0' = -a0; g' = -g; e2' = A0 u' = -e2
  hvv = -2 sum(e1) + 2 sum(e2');  num = hvv - g'.p';  out = p' + num/(1+gg) * g'

Layout: features on partitions, batch on the free axis, tiles of 512.
The per-element scalar "tail" (dot products, Sherman-Morrison scale) runs
batch-major after a PE transpose of the packed [force; g; hvv] block.
All big matmuls use float32r (1 cycle/row vs 4 for fp32).

Sharding: pure data parallel over 8 NeuronCores (8192 rows each), weights
replicated, outputs concatenated.
"""

import os

import ml_dtypes
import numpy as np

import concourse.bacc as bacc
import concourse.bass as bass
import concourse.dve_ops as dve_ops
import concourse.tile as tile
from concourse import mybir
from concourse.bass_utils import run_bass_kernel_spmd
from concourse.dve_ops import DveOp
from concourse.dve_ops import has_src1
from concourse.dve_spec import C0, C1, One, Spec, Src0, Src1, lower, sq
from concourse.dve_uop import DveOpSpec
from concourse.masks import make_identity

F32 = mybir.dt.float32
F32R = mybir.dt.float32r
BF16 = mybir.dt.bfloat16
F16 = mybir.dt.float16
AX = mybir.AxisListType
OP = mybir.AluOpType
ACT = mybir.ActivationFunctionType

DIM = 8
H = 128
BATCH = 65536
NCORES = 8
BC = BATCH // NCORES          # 8192 rows per core
TW = 512                      # batch tile width
NT = BC // TW                 # 16 tiles per core
NCH = TW // 128               # 4 chunks of 128 per tile
JPC = BC // 128               # 64 column-groups in X_sb

LAST_RESULTS = None

# ---------------- custom fused DVE ops ----------------


def _register_op(name, body, reference):
    if name in dve_ops._SUB_OPCODE_FOR_NAME:
        for op in dve_ops.OPS:
            if op.name == name:
                return op
    spec = Spec(body=body, reference=reference)
    shas = {}
    for ver in ("v3", "v4"):
        shas[ver] = DveOpSpec(
            name=name,
            opcode=dve_ops._CUSTOM_DVE_ROW_BASE + len(dve_ops.OPS),
            uops=lower(spec, ver=ver),
            rd1_en=has_src1(spec),
        ).sha(ver)
    op = DveOp(name, spec, subdim=False, uops_sha=shas)
    dve_ops.OPS.append(op)
    dve_ops.CUSTOM_DVE_SPECS[name] = spec
    dve_ops._SUB_OPCODE_FOR_NAME[name] = (
        dve_ops._CUSTOM_DVE_ROW_BASE + len(dve_ops.OPS) - 1
    )
    return op


# h0p' = (h0^2 - 1) * t0     (also a0' = (h0^2 - 1) * A0)
OP_SQM1_MUL = _register_op(
    "ANT_SQM1_MUL",
    (sq(Src0) - One) * Src1,
    lambda in0, in1: (in0 * in0 - 1.0) * in1,
)
# u' = h0 * (h0^2 - 1) * t0^2
OP_UPRIME = _register_op(
    "ANT_UPRIME",
    Src0 * (sq(Src0) - One) * sq(Src1),
    lambda in0, in1: in0 * (in0 * in0 - 1.0) * in1 * in1,
)
# e1 = (1 - h1^2) * w2 * h1 * t1^2
OP_E1F = _register_op(
    "ANT_E1F",
    (One - sq(Src0)) * C0 * Src0 * sq(Src1),
    lambda in0, in1, s0: (1.0 - in0 * in0) * s0 * in0 * in1 * in1,
)
# a1 = (1 - h1^2) * w2
OP_A1F = _register_op(
    "ANT_A1F",
    (One - sq(Src0)) * C0,
    lambda in0, s0: (1.0 - in0 * in0) * s0,
)
# out = p' + (num * rec) * g'
OP_OUTF = _register_op(
    "ANT_OUTF",
    Src0 + (Src1 * C0) * C1,
    lambda in0, in1, s0, s1: in0 + (s0 * s1) * in1,
)


def build_nc():
    nc = bacc.Bacc()

    X = nc.dram_tensor("X", [BC, 2 * DIM], F32, kind="ExternalInput")
    # host-preprocessed weights (f32r where consumed by f32r matmuls)
    W0r = nc.dram_tensor("W0r", [H, 32], F16, kind="ExternalInput")
    W0Tx = nc.dram_tensor("W0Tx", [2 * DIM, H], F16, kind="ExternalInput")
    W0Tv = nc.dram_tensor("W0Tv", [2 * DIM, H], F16, kind="ExternalInput")
    W1 = nc.dram_tensor("W1", [H, H], F16, kind="ExternalInput")
    W1T = nc.dram_tensor("W1T", [H, H], F16, kind="ExternalInput")
    KDTn = nc.dram_tensor("KDTn", [2 * DIM, DIM], F16, kind="ExternalInput")
    b0c = nc.dram_tensor("b0c", [H, 1], F32, kind="ExternalInput")
    b1c = nc.dram_tensor("b1c", [H, 1], F32, kind="ExternalInput")
    w2c = nc.dram_tensor("w2c", [H, 1], F32, kind="ExternalInput")
    out = nc.dram_tensor("out", [BC, DIM], F32, kind="ExternalOutput")

    from contextlib import ExitStack

    with tile.TileContext(nc) as tc, ExitStack() as stk:
        consts = stk.enter_context(tc.tile_pool(name="consts", bufs=1))
        work = stk.enter_context(tc.tile_pool(name="work", bufs=2))
        ps1 = stk.enter_context(tc.tile_pool(name="ps1", bufs=1, space="PSUM"))
        ps2 = stk.enter_context(tc.tile_pool(name="ps2", bufs=2, space="PSUM"))
        psT = ps1

        # ---------------- constants ----------------
        ident = consts.tile([128, 128], F32)
        make_identity(nc, ident)
        ident_r = consts.tile([128, 128], F32R)
        nc.scalar.copy(ident_r, ident)

        X_sb = consts.tile([128, JPC * 16], F32)
        nc.sync.dma_start(out=X_sb, in_=X.rearrange("(p j) f -> p (j f)", p=128))
        X_sr = consts.tile([128, JPC * 16], F32R)
        nc.scalar.copy(X_sr, X_sb)

        W0_sb = consts.tile([H, 32], F16)
        nc.sync.dma_start(out=W0_sb, in_=W0r[:, :])
        W0Tx_sb = consts.tile([2 * DIM, H], F16)
        nc.sync.dma_start(out=W0Tx_sb, in_=W0Tx[:, :])
        W0Tv_sb = consts.tile([2 * DIM, H], F16)
        nc.sync.dma_start(out=W0Tv_sb, in_=W0Tv[:, :])
        W1_sb = consts.tile([H, H], F16)
        nc.sync.dma_start(out=W1_sb, in_=W1[:, :])
        W1T_sb = consts.tile([H, H], F16)
        nc.sync.dma_start(out=W1T_sb, in_=W1T[:, :])
        KDTn_sb = consts.tile([2 * DIM, DIM], F16)
        nc.sync.dma_start(out=KDTn_sb, in_=KDTn[:, :])
        b0_sb = consts.tile([H, 1], F32)
        nc.sync.dma_start(out=b0_sb, in_=b0c[:, :])
        b1_sb = consts.tile([H, 1], F32)
        nc.sync.dma_start(out=b1_sb, in_=b1c[:, :])
        w2_sb = consts.tile([H, 1], F32)
        nc.sync.dma_start(out=w2_sb, in_=w2c[:, :])

        # hvv reduction vectors: +-2 in cols 0:8, zeros in cols 8:32
        m2o8 = consts.tile([H, 32], F32)
        nc.vector.memset(m2o8, 0.0)
        nc.vector.memset(m2o8[:, 0:DIM], -2.0)
        m2o8_r = consts.tile([H, 32], F16)
        nc.scalar.copy(m2o8_r, m2o8)
        p2o8_r = consts.tile([H, 32], F16)
        nc.scalar.mul(p2o8_r, m2o8, -1.0)

        out_sb = consts.tile([128, JPC * DIM], F32)

        # ---------------- main loop ----------------
        for t in range(NT):
            # transpose 4 chunks of X into XT [16, 512] (features x batch)
            xt_ps = ps1.tile([2 * DIM, TW], F32R, tag="xt")
            for c in range(NCH):
                j = NCH * t + c
                nc.tensor.transpose(
                    xt_ps[:, c * 128 : (c + 1) * 128],
                    X_sr[:, 16 * j : 16 * (j + 1)],
                    ident_r,
                )
            XT = work.tile([2 * DIM, TW], F16)
            nc.scalar.copy(XT, xt_ps.bitcast(F32))

            z0 = ps2.tile([H, TW], F32, tag="zz")
            nc.tensor.matmul(z0, W0Tx_sb, XT, start=True, stop=True)
            t0 = ps2.tile([H, TW], F32, tag="tt")
            nc.tensor.matmul(t0, W0Tv_sb, XT, start=True, stop=True)

            h0 = work.tile([H, TW], F16)
            nc.scalar.activation(h0, z0, ACT.Tanh, bias=b0_sb, scale=1.0)
            h0f = h0

            # h0p' = (h0^2-1)*t0 ; u' = h0*(h0^2-1)*t0^2
            h0p = work.tile([H, TW], F16)
            nc.vector._custom_dve(OP_SQM1_MUL, out=h0p, in0=h0f, in1=t0[:, :])
            u = work.tile([H, TW], F32)
            nc.vector._custom_dve(OP_UPRIME, out=u, in0=h0f, in1=t0[:, :])

            z1 = ps2.tile([H, TW], F32, tag="zz")
            nc.tensor.matmul(z1, W1T_sb, h0, start=True, stop=True)
            t1 = ps2.tile([H, TW], F32, tag="tt")
            nc.tensor.matmul(t1, W1T_sb, h0p, start=True, stop=True)

            h1 = work.tile([H, TW], F32R)
            nc.scalar.activation(h1, z1, ACT.Tanh, bias=b1_sb, scale=1.0)
            h1f = h1.bitcast(F32)

            # a1 = (1-h1^2)*w2 ; e1 = a1*h1*t1^2
            a1 = work.tile([H, TW], F16)
            nc.vector._custom_dve(OP_A1F, out=a1, in0=h1f, s0=w2_sb[:, 0:1])
            e1 = work.tile([H, TW], F16)
            nc.vector._custom_dve(
                OP_E1F, out=e1, in0=h1f, in1=t1[:, :], s0=w2_sb[:, 0:1]
            )

            A0 = ps1.tile([H, TW], F32, tag="A0")
            nc.tensor.matmul(A0, W1_sb, a1, start=True, stop=True)

            # a0' = (h0^2-1)*A0 ; e2' = A0*u'
            a0 = work.tile([H, TW], F16)
            nc.vector._custom_dve(OP_SQM1_MUL, out=a0, in0=h0f, in1=A0[:, :])
            e2 = work.tile([H, TW], F16)
            nc.vector.tensor_mul(e2, A0, u)

            # feature-major tail: p' rows 0:8, g' rows 32:40, hvv rows 64:72
            fm = psT.tile([96, TW], F32, tag="fm")
            nc.tensor.matmul(fm[0:DIM, :], KDTn_sb, XT, start=True, stop=True)
            nc.tensor.matmul(fm[32:64, :], W0_sb, a0, start=True, stop=True, tile_position=(0, 32))
            nc.tensor.matmul(fm[64:96, :], m2o8_r, e1, start=True, stop=False, tile_position=(0, 64))
            nc.tensor.matmul(fm[64:96, :], p2o8_r, e2, start=False, stop=True, tile_position=(0, 64))

            E = work.tile([72, TW], F32R)
            nc.scalar.copy(E, fm[0:72, :])

            # transpose to batch-major: bm[:, 72c + k] = E[k, 128c + p]
            bm = psT.tile([128, NCH * 72], F32R, tag="bm")
            for c in range(NCH):
                nc.tensor.transpose(
                    bm[:, 72 * c : 72 * (c + 1)],
                    E[:, c * 128 : (c + 1) * 128],
                    ident_r[0:72, 0:72],
                )
            # pack [p'(8) g'(8) hv(8)] per chunk: src rows 0:8,32:40,64:72
            tl = work.tile([128, NCH * 24], F32)
            bmf = bm.bitcast(F32)
            src4 = bass.AP(
                tensor=bmf.tensor,
                offset=bmf.offset,
                ap=[list(bmf.ap[0]), [72, NCH], [32, 3], [1, DIM]],
            )
            nc.scalar.copy(
                tl.rearrange("p (c q f) -> p c q f", q=3, f=DIM), src4
            )

            def col3(off, w):
                return bass.AP(
                    tensor=tl.tensor,
                    offset=tl.offset + off,
                    ap=[list(tl.ap[0]), [24, NCH], [1, w]],
                )

            p3 = col3(0, DIM)
            g3 = col3(DIM, DIM)
            hv2 = bass.AP(
                tensor=tl.tensor,
                offset=tl.offset + 2 * DIM,
                ap=[list(tl.ap[0]), [24, NCH]],
            )

            gb = work.tile([128, 2 * NCH * DIM], F32)
            gb3 = gb.rearrange("p (q c f) -> p (q c) f", f=DIM, q=2)
            nc.vector.tensor_mul(
                gb3[:, 0:NCH, :].rearrange("p c f -> p c f"), g3, g3
            )
            nc.vector.tensor_mul(
                gb3[:, NCH : 2 * NCH, :].rearrange("p c f -> p c f"), g3, p3
            )
            red = work.tile([128, 2 * NCH], F32)
            nc.vector.tensor_reduce(red, gb3, axis=AX.X, op=OP.add)
            gg = red[:, 0:NCH]
            gps = red[:, NCH : 2 * NCH]
            den = work.tile([128, NCH], F32)
            nc.vector.tensor_scalar_add(den, gg, 1.0)
            rec = work.tile([128, NCH], F32)
            nc.vector.reciprocal(rec, den)
            num = work.tile([128, NCH], F32)
            nc.vector.tensor_sub(num, hv2, gps)
            s4 = work.tile([128, NCH], F32)
            nc.vector.tensor_mul(s4, num, rec)
            s4b = bass.AP(
                tensor=s4.tensor,
                offset=s4.offset,
                ap=[list(s4.ap[0]), [1, NCH], [0, DIM]],
            )
            su = work.tile([128, NCH * DIM], F32)
            su3 = su.rearrange("p (c f) -> p c f", f=DIM)
            nc.vector.tensor_mul(su3, g3, s4b)
            ob = out_sb[:, DIM * NCH * t : DIM * NCH * (t + 1)]
            nc.vector.tensor_add(
                ob.rearrange("p (c f) -> p c f", f=DIM), p3, su3
            )

        nc.sync.dma_start(
            out=out.rearrange("(p j) f -> p (j f)", p=128), in_=out_sb
        )

    if not nc.is_finalized():
        nc.finalize()

    return nc


_NC_CACHE = None


def _install_ntff_shim():
    """Register the axon NTFF profile hook (missing antenv.axon_hooks shim)."""
    import sys
    import types

    if "antenv.axon_hooks" in sys.modules:
        return
    try:
        sys.path.insert(0, "/root/.axon_site")
        from trn_agent_boot.trn_boot import _ntff_profile_via_ctypes

        hook = _ntff_profile_via_ctypes("/opt/axon/libaxon_pjrt.so")
        mod = types.ModuleType("antenv.axon_hooks")
        mod.get_axon_ntff_profile_hook = lambda: hook
        sys.modules["antenv.axon_hooks"] = mod
    except Exception:
        pass


def kernel(**inputs):
    global LAST_RESULTS, _NC_CACHE
    trace = bool(int(os.environ.get("KERNEL_TRACE", "0")))
    if trace:
        _install_ntff_shim()
    if _NC_CACHE is None:
        _NC_CACHE = build_nc()
    nc = _NC_CACHE

    X = np.ascontiguousarray(inputs["X"], dtype=np.float32)
    K = np.asarray(inputs["K"], np.float32)
    D = np.asarray(inputs["D"], np.float32)
    W0 = np.asarray(inputs["W0"], np.float32)
    W1 = np.asarray(inputs["W1"], np.float32)
    W2 = np.asarray(inputs["W2"], np.float32)
    w0pad = np.zeros((H, 32), np.float32)
    w0pad[:, 0:DIM] = W0
    w0tx = np.zeros((2 * DIM, H), np.float32)
    w0tx[0:DIM] = W0.T
    w0tv = np.zeros((2 * DIM, H), np.float32)
    w0tv[DIM:] = W0.T
    shared = {
        "W0r": w0pad.astype(np.float16),
        "W0Tx": w0tx.astype(np.float16),
        "W0Tv": w0tv.astype(np.float16),
        "W1": W1.astype(np.float16),
        "W1T": np.ascontiguousarray(W1.T).astype(np.float16),
        "KDTn": np.ascontiguousarray(np.concatenate([-K.T, -D.T], axis=0)).astype(np.float16),
        "b0c": np.asarray(inputs["b0"], np.float32).reshape(H, 1).copy(),
        "b1c": np.asarray(inputs["b1"], np.float32).reshape(H, 1).copy(),
        "w2c": W2.reshape(H, 1).copy(),
    }
    in_maps = []
    for i in range(NCORES):
        m = {"X": X[i * BC : (i + 1) * BC]}
        m.update(shared)
        in_maps.append(m)

    res = run_bass_kernel_spmd(
        nc, in_maps, core_ids=list(range(NCORES)), trace=trace
    )
    LAST_RESULTS = res
    out_full = np.concatenate([res.results[i]["out"] for i in range(NCORES)], axis=0)
    return out_full.astype(np.float32)
